# revision 13
# baseline (speedup 1.0000x reference)
"""DepthCueExtractor TRN2 kernel.

out[b,u,y,x,n] = mean_v(lfi[b,u,y,x,v]) * s_mask[b,n] * h_mask[b,n,y]
  s_mask[b,n]   = sum_{h,w} f_maps[b,h,w,n]
  h_mask[b,n,y] = colsum[b,y,n] / max_w colsum[b,w,n]
  colsum[b,w,n] = sum_h f_maps[b,h,w,n]

Sharding: 8 cores = (batch b in 0..3) x (H-half in 0..1), data-parallel on the
output. Memory-bound problem, so all large traffic is in reduced precision:
lfi and the output travel as fp16 (output upcast to f32 on host), f_maps as
fp8e4m3 (it only feeds smooth sum/max reductions). Each core reads the FULL
f_maps[b] (4.2MB at fp8, permuted "my w-half first" on the host) so the
global sum/max stats are local and no collective is needed at all.

colsum is computed with PE ones-matmuls accumulating both h-halves directly
in PSUM. The output phase writes n-major [U, HY, N, W] tiles (host transposes
back) so each (u, n) slice is a per-partition-scalar multiply
  ot[y, n, :] = mlf_u[y, :] * wf[y, n]
i.e. InstTensorScalarPtr with packed fp16 operands -> 4x DVE mode. A 1/4
scale is folded into wf to keep fp16 products below 65504; the host multiplies
the final f32 output by 4. ~47MB of HBM traffic per core, ~360GB/s roofline.
"""

import numpy as np

import concourse.bass as bass
import concourse.bacc as bacc
import concourse.bass_isa as bass_isa
import concourse.mybir as mybir
import concourse.tile as tile
from concourse.bass_utils import run_bass_kernel_spmd

F32 = mybir.dt.float32
F16 = mybir.dt.float16
F8 = mybir.dt.float8e4

NP_F16 = mybir.dt.np(F16)
NP_F8 = mybir.dt.np(F8)

B, U, H, W, V, N = 4, 9, 256, 256, 9, 64
HY = H // 2
SCALE = 4.0  # folded out of wf to keep fp16 products in range


def build_kernel_body(nc, tc, lfi_s, fm, out_s):
    with (
        tc.tile_pool(name="const", bufs=1) as const_pool,
        tc.tile_pool(name="fmp", bufs=4) as fm_pool,
        tc.tile_pool(name="psum", bufs=1, space="PSUM") as psum_pool,
        tc.tile_pool(name="stats", bufs=1) as stats_pool,
        tc.tile_pool(name="lfip", bufs=1) as lfi_pool,
        tc.tile_pool(name="mlfp", bufs=1) as mlf_pool,
        tc.tile_pool(name="outp", bufs=3) as out_pool,
    ):
        ones = const_pool.tile([128, 1], F8)
        nc.vector.memset(ones[:], 1.0)

        # lt0 loads before fm so reduce_0 is done long before wf is ready;
        # the first output tile then only waits on the (fm-bound) stats chain.
        lfi_tiles = {}

        def load_u(u):
            lt = lfi_pool.tile([128, W, V], F16, name=f"lt{u}", tag=f"lt{u}")
            nc.sync.dma_start(out=lt[:], in_=lfi_s[u])
            lfi_tiles[u] = lt

        load_u(0)

        # ---- Phase A: colsum[w, n] = sum_h fm[h, w, n] for all 256 w.
        # fm is laid out my-w-half-first, so wq=0 is this core's half. One
        # single-shot PSUM tile per (wq, h-half) — no PSUM accumulation
        # groups (start/stop accumulation across matmuls proved unreliable
        # on HW); the h-halves are added on DVE in the stats phase.
        cs_psum = {}
        for ht in range(2):
            for wq in range(2):
                cs_psum[wq, ht] = psum_pool.tile([128, N], F32, name=f"cs{wq}{ht}")
                ft = fm_pool.tile(
                    [128, 128, N], F8, name=f"f{ht}_{wq}", tag="fm", bufs=4
                )
                nc.sync.dma_start(
                    out=ft[:],
                    in_=fm[ht * 128 : (ht + 1) * 128, wq * 128 : (wq + 1) * 128, :],
                )
                for n in range(N):
                    nc.tensor.matmul(
                        out=cs_psum[wq, ht][:, n : n + 1],
                        lhsT=ft[:, :, n],
                        rhs=ones[:, 0:1],
                        start=True,
                        stop=True,
                    )

        # ---- Phase B head: queue the remaining lfi loads right after fm.
        for u in range(1, U):
            load_u(u)

        mlf = [
            mlf_pool.tile([128, W], F16, name=f"mlf{u}", tag=f"mlf{u}")
            for u in range(U)
        ]

        acc = [
            mlf_pool.tile([128, W], F32, name=f"acc{u}", tag=f"acc{u % 2}")
            for u in range(U)
        ]

        def reduce_u(u):
            # V-sum as chained adds on GPSIMD so DVE stays a pure TSP stream.
            # f32 accumulator; only the final add rounds to fp16 (~2^-11).
            lt, a = lfi_tiles[u], acc[u]
            with nc.allow_low_precision(reason="fp16 V-sum, f32 accumulator"):
                nc.gpsimd.tensor_add(
                    out=a[:], in0=lt[:, :, 0], in1=lt[:, :, 1]
                )
                for v in range(2, V - 1):
                    nc.gpsimd.tensor_add(out=a[:], in0=a[:], in1=lt[:, :, v])
                nc.gpsimd.tensor_add(
                    out=mlf[u][:], in0=a[:], in1=lt[:, :, V - 1]
                )

        reduce_u(0)

        # ---- Phase A2: local stats over both halves -> wf[y, n].
        hp = tc.high_priority
        with hp():
            # only one non-scalar PSUM input allowed per DVE op: copy one
            # h-half to SBUF, then add the other PSUM half onto it.
            cs_sb = stats_pool.tile([128, N], F32)
            nc.vector.tensor_copy(out=cs_sb[:], in_=cs_psum[0, 0][:])
            nc.vector.tensor_add(
                out=cs_sb[:], in0=cs_sb[:], in1=cs_psum[0, 1][:]
            )
            cs_ob = stats_pool.tile([128, N], F32)
            nc.vector.tensor_copy(out=cs_ob[:], in_=cs_psum[1, 0][:])
            nc.vector.tensor_add(
                out=cs_ob[:], in0=cs_ob[:], in1=cs_psum[1, 1][:]
            )

            red = []
            for si, src in enumerate((cs_sb, cs_ob)):
                for oi, op in enumerate((bass_isa.ReduceOp.add, bass_isa.ReduceOp.max)):
                    r = stats_pool.tile([128, N], F32, name=f"red{si}{oi}")
                    nc.gpsimd.partition_all_reduce(r[:], src[:], 128, op)
                    red.append(r)

            s_all = stats_pool.tile([128, N], F32)
            nc.vector.tensor_add(out=s_all[:], in0=red[0][:], in1=red[2][:])
            m_all = stats_pool.tile([128, N], F32)
            nc.vector.tensor_max(out=m_all[:], in0=red[1][:], in1=red[3][:])
            mve = stats_pool.tile([128, N], F32)
            nc.vector.tensor_scalar_mul(mve[:], m_all[:], float(V) * SCALE)
            rec = stats_pool.tile([128, N], F32)
            nc.vector.reciprocal(out=rec[:], in_=mve[:])
            sn = stats_pool.tile([128, N], F32)
            nc.vector.tensor_mul(out=sn[:], in0=s_all[:], in1=rec[:])
            wf = stats_pool.tile([128, N], F32)
            nc.vector.tensor_mul(out=wf[:], in0=cs_sb[:], in1=sn[:])

        # ---- Phase C: ot[y, n, x] = mlf_u[y, x] * wf[y, n] via per-partition
        # scalar multiplies (4x DVE mode), streamed to HBM n-major.
        for u in range(U):
            ot = out_pool.tile([128, N, W], F16, name=f"ot{u}", tag="ot", bufs=3)
            for n in range(N):
                nc.vector.tensor_scalar_mul(
                    ot[:, n, :], mlf[u][:, :], wf[:, n : n + 1]
                )
            nc.sync.dma_start(out=out_s[u], in_=ot[:])
            if u + 1 < U:
                reduce_u(u + 1)


def build_nc():
    nc = bacc.Bacc("TRN2", target_bir_lowering=False, debug=True)
    lfi_s = nc.dram_tensor("lfi_s", [U, HY, W, V], F16, kind="ExternalInput")
    fm = nc.dram_tensor("fm", [H, W, N], F8, kind="ExternalInput")
    out_s = nc.dram_tensor("out_s", [U, HY, N, W], F16, kind="ExternalOutput")
    with tile.TileContext(nc) as tc:
        build_kernel_body(nc, tc, lfi_s, fm, out_s)
    nc.compile()
    return nc


_CACHE = {}


def make_in_maps(lfi, f_maps):
    lfi16 = lfi.astype(NP_F16)
    fm8 = f_maps.astype(NP_F8)
    in_maps = []
    for c in range(8):
        b, half = divmod(c, 2)
        lf = np.ascontiguousarray(lfi16[b, :, half * HY : (half + 1) * HY])
        # my w-half first, partner's half second
        fmc = np.concatenate(
            [
                fm8[b][:, half * HY : (half + 1) * HY, :],
                fm8[b][:, (1 - half) * HY : (2 - half) * HY, :],
            ],
            axis=1,
        )
        in_maps.append({"lfi_s": lf, "fm": np.ascontiguousarray(fmc)})
    return in_maps


def kernel(lfi, f_maps):
    lfi = np.asarray(lfi, dtype=np.float32)
    f_maps = np.asarray(f_maps, dtype=np.float32)
    if "nc" not in _CACHE:
        _CACHE["nc"] = build_nc()
    nc = _CACHE["nc"]
    res = run_bass_kernel_spmd(nc, make_in_maps(lfi, f_maps), list(range(8)))
    out = np.empty((B, U, H, W, N), np.float32)
    for c in range(8):
        b, half = divmod(c, 2)
        o = res.results[c]["out_s"].astype(np.float32) * SCALE  # [U, HY, N, W]
        out[b, :, half * HY : (half + 1) * HY] = o.transpose(0, 1, 3, 2)
    return out
